# revision 44
# baseline (speedup 1.0000x reference)
"""Trainium2 Bass kernel for CustomMultiHeadSelfAttention (fused q/k LayerNorm).

Reference computation (per batch n):
    q = x @ Wq.T ; k = x @ Wk.T ; v = x @ Wv.T          (split into 16 heads of 64)
    q = LN_head(q) * gq + bq ; k = LN_head(k) * gk + bk  (LayerNorm over head_dim)
    out = causal_softmax(q @ k.T) @ v                    (per head)
    y = concat_heads(out) @ Wo.T + bo

Sharding: 8 cores = 2 batches x 4 head-groups (4 heads each).  Each core
computes its heads' attention and a partial y = out_heads @ Wo[:, cols].T;
the host sums the 4 partials per batch and adds bo.

Device dataflow (all 16-bit operands fp16 except P=exp(S) which is bf16 for
range; PSUM accumulation fp32):
  - Projections stream fp16 xT/W chunks; LN mean is folded into centered
    Q/K weights so LN reduces to q * rsqrt(mean(q^2)+eps); rstd is
    broadcast per head-group on GPSIMD and applied with one tensor_tensor;
    *g+b is fused into the post-transpose PSUM->SBUF store.
  - Scores are computed transposed per head, ST[m, l] = k_ln q_ln^T, with a
    fp16 -3e4 additive mask on the diagonal 128x128 block via an identity
    matmul; exp() on ScalarE (no max subtraction: LN bounds |score| <= 64).
  - Overlap: for m-blocks j<8 the l<1024 part of the scores ("window") is
    emitted between the two projection halves so ScalarE exps run under the
    second projection half; P tiles are stashed in SBUF.
  - O is accumulated P-stationary per l-tile: O[l,65] += P[m,l-block]^T
    V_aug[m,65] (V has a ones column -> col 64 = softmax denominator), so
    the moving operand is only 65 wide.  Slot t is normalized (per-partition
    reciprocal) as soon as j reaches t, transposed back to OT layout on the
    PE, and written to the fc-out stationary tile.
  - Output projection y[t] = sum_p OT_p[:,t]^T @ WoT_p is emitted per
    l-tile inside the last head's pass, so it overlaps attention.
"""

import numpy as np

import concourse.bass as bass
import concourse.tile as tile
from concourse import bacc, mybir
from concourse.bass_utils import run_bass_kernel_spmd

F32 = mybir.dt.float32
F32R = mybir.dt.float32r
F16 = mybir.dt.float16
BF16 = mybir.dt.bfloat16
F8E5 = mybir.dt.float8e5
U8 = mybir.dt.uint8
DR = mybir.MatmulPerfMode.DoubleRow

P = 128
EMB = 1024
L = 2048
D = 64
HPC = 4           # heads per core
NCORES = 8
EPS = 1e-5
T = L // P        # 16 l-tiles
E = EMB // P      # 8 emb chunks
WJ = 8            # window m-blocks: j<8 cover l in [128j, 1024)
WEND = 1024
USE_DR_MASK = False
AF = mybir.ActivationFunctionType
ALU = mybir.AluOpType


def _subs(off, ln):
    """Split [off, off+ln) at 512-multiples (PSUM bank boundaries)."""
    out = []
    cur = off
    while cur < off + ln:
        nxt = min((cur // 512 + 1) * 512, off + ln)
        out.append((cur, nxt - cur))
        cur = nxt
    return out


def build_nc():
    nc = bacc.Bacc("TRN2", target_bir_lowering=False, debug=False, num_devices=NCORES)

    xT_d = nc.dram_tensor("xT", [E, P, L], F16, kind="ExternalInput")
    wqk_d = nc.dram_tensor("wqk", [E, P, 512], F16, kind="ExternalInput")
    wv_d = nc.dram_tensor("wv", [E, P, 256], F16, kind="ExternalInput")
    wo_d = nc.dram_tensor("wo", [P, 2, EMB], F16, kind="ExternalInput")
    im_d = nc.dram_tensor("im", [P, 2, P], F16, kind="ExternalInput")  # ident|mask
    gb_d = nc.dram_tensor("gb", [P, 4], F32, kind="ExternalInput")  # gq2 bq2 gk2 bk2
    y_d = nc.dram_tensor("y", [L, EMB], F16, kind="ExternalOutput")

    with tile.TileContext(nc) as tc:
        with (
            tc.tile_pool(name="const", bufs=1) as const_p,
            tc.tile_pool(name="vbuf", bufs=1) as vbuf_p,
            tc.tile_pool(name="qtkt", bufs=1) as qtkt_p,
            tc.tile_pool(name="stash", bufs=1) as stash_p,
        ):
            im = const_p.tile([P, 2, P], F16, tag="im")
            gb = const_p.tile([P, 4], F32, tag="gb")
            epst = const_p.tile([P, 1], F32, tag="epst")
            nc.vector.memset(epst[:], EPS)
            ident = im[:, 0, :]
            maskf = im[:, 1, :]

            # V with a ones column per head: vb[t][:, h, 0:65]
            vb = []
            for t in range(T):
                v_ = vbuf_p.tile([P, HPC, 65], BF16, tag=f"vb{t}", name=f"vb{t}")
                # only col 64 (the denominator ones-column) needs the memset;
                # DVE so the Pool DGE queue stays free for the input DMAs
                nc.vector.memset(v_[:, :, 64:65], 1.0)
                vb.append(v_)

            # QT/KT: partitions = d of head pair (rows [0:64]=head 2p, [64:128]
            # = head 2p+1); dims [pair, l]
            qt = qtkt_p.tile([P, 2, L], F16, tag="qt", name="qt")
            kt = qtkt_p.tile([P, 2, L], F16, tag="kt", name="kt")

            # persistent P stash: ps[h][j] covers l in [128j, 2048) at col
            # l-128j.  j<8 tiles live for the whole kernel (their l<1024
            # window part is exp'd during phase 1); j>=8 tiles are allocated
            # inside the phase-2 pool block to keep the phase-1 footprint
            # under the SBUF limit.
            ps = [[stash_p.tile([P, L - 128 * j], BF16, tag=f"ps{h}_{j}",
                                name=f"ps{h}_{j}") for j in range(WJ)] + [None] * (T - WJ)
                  for h in range(HPC)]

            def scores_exp(pool, width, p_, hl, j, off, ln, diag, out16, ocol):
                """ST[m, l] for m-block j over l in [off, off+ln), exp into
                out16[:, ocol:ocol+ln]."""
                rows = slice(64 * hl, 64 * hl + 64)
                base = (off // width) * width if width == 512 else 512 * (off // 512)
                sps = pool.tile([P, width], F32, tag="s")
                for (soff, sln) in _subs(off, ln):
                    d_ = diag and soff == off
                    nc.tensor.matmul(
                        sps[:, soff - base:soff - base + sln],
                        kt[rows, p_, j * P:(j + 1) * P],
                        qt[rows, p_, soff:soff + sln],
                        start=True, stop=not d_)
                    if d_:
                        nc.tensor.matmul(
                            sps[:, off - base:off - base + P],
                            ident, maskf, start=False, stop=True)
                nc.scalar.activation(
                    out16[:, ocol:ocol + ln],
                    sps[:, off - base:off - base + ln], AF.Exp)

            # ============ Phase 1: projections + LN + transpose ==========
            # Q|K projections run first (the LN/transpose chain and the
            # attention window depend on them); V projections are deferred
            # to the back half and past the phase boundary, where they give
            # the PE useful filler while the window/first main exps run.
            with (
                tc.tile_pool(name="xt", bufs=1) as xt_p,
                tc.tile_pool(name="wts", bufs=1) as wts_p,
                tc.tile_pool(name="rows", bufs=4) as rows_p,
                tc.tile_pool(name="stats", bufs=4) as stats_p,
                tc.tile_pool(name="sq", bufs=3) as sq_p,
                tc.tile_pool(name="rsb", bufs=3) as rsb_p,
                tc.tile_pool(name="ps_qk", bufs=3, space="PSUM") as ps_qk,
                tc.tile_pool(name="ps_v", bufs=1, space="PSUM") as ps_v,
                tc.tile_pool(name="ps_tr", bufs=1, space="PSUM") as ps_tr,
                tc.tile_pool(name="ps_sw", bufs=2, space="PSUM") as ps_sw,
            ):
                # e-chunked loads round-robined over the SP / ACT / gpsimd
                # DGE queues: parallel issue + parallel DMA engines so the
                # first qk-projection step can start ~2.5us in
                wqk = wts_p.tile([P, E, 512], F16, tag="wqk", name="wqk")
                wv = wts_p.tile([P, E, 256], F16, tag="wv", name="wv")
                xq = [xt_p.tile([P, E, 512], F16, tag=f"xq{q}", name=f"xq{q}")
                      for q in range(4)]
                for ep in range(4):
                    es = slice(2 * ep, 2 * ep + 2)
                    nc.sync.dma_start(
                        wqk[:, es, :], wqk_d[es].rearrange("e p c -> p e c"))
                    nc.scalar.dma_start(
                        xq[0][:, es, :],
                        xT_d[es, :, 0:512].rearrange("e p c -> p e c"))
                nc.sync.dma_start(im[:], im_d[:])
                nc.sync.dma_start(gb[:], gb_d[:])
                for q in range(1, 4):
                    eng = (nc.gpsimd, nc.sync, nc.scalar)[q - 1]
                    eng.dma_start(
                        xq[q][:], xT_d[:, :, 512 * q:512 * (q + 1)].rearrange(
                            "e p c -> p e c"))
                nc.gpsimd.dma_start(wv[:], wv_d.rearrange("e p c -> p e c"))

                # p-state warmup: ~26 junk matmuls keep the PE continuously
                # busy from ~0.4us until the first input chunks land, so the
                # first projection (and everything after) runs at 2.4 GHz
                # instead of spending its first 3us at 1.2.
                warm_sb = rows_p.tile([P, 256], F16, tag="warm", name="warm")
                nc.vector.memset(warm_sb[:], 0.0)
                warm_ps = ps_sw.tile([P, 512], F32, tag="s", name="warm_ps")
                for _ in range(18):
                    nc.tensor.matmul(warm_ps[:, 0:256], warm_sb[:, 0:128],
                                     warm_sb[:], start=True, stop=True)

                pvt = ps_v.tile([P, 2, 256], F32, tag="pv")
                trt = ps_tr.tile([P, 2, 2, 2, P], F16, tag="tr")  # [t%2,qk,pair]
                rowt = [None] * T  # (qrow, krow) awaiting lag-2 transpose
                chain = [None] * T  # (pqk, rsb) awaiting lag-1 rstd-mult

                def proj_flush(t):
                    # transposes + qt/kt stores for step t (*g+b fused)
                    qrow, krow = rowt[t]
                    s = t % 2
                    for p_ in range(2):
                        nc.tensor.transpose(
                            trt[:, s, 0, p_, :],
                            qrow[:, 128 * p_:128 * (p_ + 1)], ident)
                        nc.tensor.transpose(
                            trt[:, s, 1, p_, :],
                            krow[:, 128 * p_:128 * (p_ + 1)], ident)
                    nc.vector.tensor_scalar(
                        qt[:, :, t * P:(t + 1) * P], trt[:, s, 0, :, :],
                        gb[:, 0:1], gb[:, 1:2], ALU.mult, ALU.add)
                    nc.vector.tensor_scalar(
                        kt[:, :, t * P:(t + 1) * P], trt[:, s, 1, :, :],
                        gb[:, 2:3], gb[:, 3:4], ALU.mult, ALU.add)

                def ln_tail(t):
                    # rstd-multiply of step t, emitted one step late so the
                    # in-order DVE queue never blocks waiting the ACT/Pool
                    # chain (which would serialize consecutive steps)
                    pqk, rsb = chain[t]
                    qrow = rows_p.tile([P, 256], F16, tag="qrow")
                    krow = rows_p.tile([P, 256], F16, tag="krow")
                    nc.vector.tensor_tensor(
                        qrow[:], pqk[:, 0:256], rsb[:, 0:256], ALU.mult)
                    nc.vector.tensor_tensor(
                        krow[:], pqk[:, 256:512], rsb[:, 256:512], ALU.mult)
                    rowt[t] = (qrow, krow)

                def proj_qk(t):
                    if t >= 2:
                        proj_flush(t - 2)
                    pqk = ps_qk.tile([P, 512], F32, tag="pqk")
                    for e in range(E):
                        xch = xq[t // 4][:, e, (t % 4) * P:(t % 4 + 1) * P]
                        nc.tensor.matmul(pqk[:], xch, wqk[:, e, :],
                                         start=(e == 0), stop=(e == E - 1))
                    # LN stats: sum of squares per (l, head) for q and k
                    sq = sq_p.tile([P, 512], F32, tag="sq")
                    nc.scalar.activation(sq[:], pqk[:], AF.Square)
                    ssq = stats_p.tile([P, 8], F32, tag="ssq")
                    nc.vector.tensor_reduce(
                        ssq[:], sq[:].rearrange("p (g d) -> p g d", d=D),
                        axis=mybir.AxisListType.X, op=ALU.add)
                    # rstd = (ssq/64 + eps)^-0.5 as exp(-0.5*ln(v)); with Ln
                    # instead of Sqrt every ACT func in the kernel (Square/
                    # Ln/Exp/Copy) lives in the natural_log_exp_and_others
                    # table set -> a single table load for the whole kernel
                    lnv = stats_p.tile([P, 8], F32, tag="lnv")
                    nc.scalar.activation(lnv[:], ssq[:], AF.Ln,
                                         bias=epst[:], scale=1.0 / D)
                    rstd = stats_p.tile([P, 8], F32, tag="rstd")
                    nc.scalar.activation(rstd[:], lnv[:], AF.Exp, scale=-0.5)
                    # broadcast rstd across each 64-col head group (GPSIMD)
                    rsb = rsb_p.tile([P, 512], F32, tag="rsb")
                    nc.gpsimd.tensor_copy(
                        rsb[:].rearrange("p (g d) -> p g d", d=D),
                        rstd[:].broadcast_to([P, 8, D]))
                    chain[t] = (pqk, rsb)
                    if t >= 1:
                        ln_tail(t - 1)

                def proj_v(t):
                    pv = pvt[:, t % 2, :]
                    for e in range(E):
                        xch = xq[t // 4][:, e, (t % 4) * P:(t % 4 + 1) * P]
                        nc.tensor.matmul(pv, xch, wv[:, e, :],
                                         start=(e == 0), stop=(e == E - 1))
                    # V -> vb[t] (strided into 65-wide head slots); DVE so
                    # the ACT queue stays exp-only here
                    nc.vector.tensor_copy(
                        vb[t][:, :, 0:64],
                        pv.rearrange("p (h d) -> p h d", h=HPC))

                # greedy window plan: after qk-step t (which flushes tile
                # t-2) qt/kt cols < 128*(t-1) exist; emit ready score
                # pieces (h, j<8, any l) so the ACT exp stream starts as
                # soon as the first tiles are flushed and never starves
                # during the V-projection tail.  Budgets approximate the
                # per-step ACT headroom next to the LN-stats chain.
                def plan_window():
                    pend = {(h, j): 128 * j
                            for h in range(HPC) for j in range(WJ)}
                    order = [(h, j) for j in range(WJ) for h in range(HPC)]
                    nsteps = T + (T - 6)
                    sched = {t: [] for t in range(nsteps)}
                    for t in range(3, nsteps):
                        limit = min(128 * (t - 1), L) if t < T else L
                        budget = 1300 if t < 10 else (1700 if t < T else 700)
                        # tier 1: the l<1024 window (unblocks nothing later
                        # but is all that exists early); tier 2: l>=1024
                        for cap in (WEND, L):
                            for (h, j) in order:
                                while budget > 0:
                                    l0 = pend[(h, j)]
                                    if l0 >= min(limit, cap):
                                        break
                                    l1 = min(limit, cap, l0 + 512,
                                             (l0 // 512 + 1) * 512)
                                    sched[t].append(
                                        (h, j, l0, l1, l0 == 128 * j))
                                    pend[(h, j)] = l1
                                    budget -= l1 - l0
                                if budget <= 0:
                                    break
                            if budget <= 0:
                                break
                    assert all(v >= WEND for v in pend.values()), pend
                    return sched, pend

                wsched, wdone = plan_window()

                def emit_pieces(t):
                    for (h, j, l0, l1, dg) in wsched.get(t, ()):
                        p_, hl = divmod(h, 2)
                        scores_exp(ps_sw, 512, p_, hl, j, l0, l1 - l0, dg,
                                   ps[h][j][:], l0 - 128 * j)

                for t in range(T):
                    proj_qk(t)
                    emit_pieces(t)
                    if t >= 10:
                        proj_v(t - 10)
                ln_tail(T - 1)
                proj_flush(T - 2)
                proj_flush(T - 1)
                # V tail interleaved with remaining window pieces: the PE
                # and ACT both stay busy across the phase boundary
                for i, tv in enumerate(range(6, T)):
                    proj_v(tv)
                    emit_pieces(T + i)

            # ================= Phase 2: attention + fc_out ===============
            # All 4 heads interleave at the j (m-block) level: per group j
            # every head emits scores+exp(j), then O/normalize for l-tile
            # j-2, pair transposes for j-3, and the SINGLE shared fc_out
            # tile j-5 (both pairs accumulate in PSUM).  This spreads the
            # DVE normalize/copy work and the fc_out evenly over the whole
            # phase instead of piling it into per-head tails.
            with (
                tc.tile_pool(name="stash2", bufs=1) as stash2_p,
                tc.tile_pool(name="otb", bufs=1) as ot_p,
                tc.tile_pool(name="wo", bufs=1) as wo_p,
                tc.tile_pool(name="osb", bufs=6) as osb_p,
                tc.tile_pool(name="nrm", bufs=8) as nrm_p,
                tc.tile_pool(name="ysb", bufs=3) as ysb_p,
                tc.tile_pool(name="ps_s", bufs=2, space="PSUM") as ps_s,
                tc.tile_pool(name="ps_o", bufs=1, space="PSUM") as ps_o,
                tc.tile_pool(name="ps_t2", bufs=1, space="PSUM") as ps_t2,
                tc.tile_pool(name="ps_y", bufs=1, space="PSUM") as ps_y,
            ):
                for h in range(HPC):
                    for j in range(WJ, T):
                        ps[h][j] = stash2_p.tile(
                            [P, L - 128 * j], BF16, tag=f"ps{h}_{j}",
                            name=f"ps{h}_{j}")
                ot = [ot_p.tile([P, L], F16, tag=f"ot{p_}", name=f"ot{p_}")
                      for p_ in range(2)]
                wo = wo_p.tile([P, 2, EMB], F16, tag="wo")
                nc.sync.dma_start(wo[:], wo_d[:])

                # PSUM: 4 banks scores (2x1024 dbl-buf), 1 bank holds the 4
                # per-head O slots (start=True zeroing only clobbers OPEN
                # accumulation groups in a bank; closed data is read-safe),
                # 1 bank transpose scratch, 2 banks fc_out double-buffer.
                obt = ps_o.tile([P, 4, 128], F32, tag="obt")
                trt2 = ps_t2.tile([P, 2, P], F16, tag="trt2")
                yp2 = ps_y.tile([P, 2, 512], F32, tag="yp2")

                def oslot(h):
                    return obt[:, h, 0:65]

                def trs(i):
                    return trt2[:, i, :]

                osbt = [[None] * T for _ in range(2)]

                def emit_o(h, t):
                    # one l-tile slot: contiguous accumulation over j<=t
                    for j in range(t + 1):
                        nc.tensor.matmul(
                            oslot(h),
                            ps[h][j][:, 128 * (t - j):128 * (t - j) + P],
                            vb[j][:, h, :], start=(j == 0), stop=(j == t))

                def norm_dve(h, t):
                    p_, hl = divmod(h, 2)
                    rec = nrm_p.tile([P, 1], F32, tag="rec")
                    nc.vector.reciprocal(rec[:], oslot(h)[:, 64:65])
                    if hl == 0:
                        osbt[p_][t] = osb_p.tile([P, P], F16, tag="osb",
                                                 name="osb")
                    nc.vector.tensor_scalar(
                        osbt[p_][t][:, 64 * hl:64 * hl + 64],
                        oslot(h)[:, 0:64], rec[:], None, ALU.mult)

                def norm_pe(p_, t):
                    nc.tensor.transpose(trs(p_), osbt[p_][t][:], ident)
                    nc.vector.tensor_copy(
                        ot[p_][:, t * P:(t + 1) * P], trs(p_))

                def emit_y(t, tail=False):
                    ysb = ysb_p.tile([P, EMB], F16, tag="ysb")
                    for c in range(2):
                        ys = yp2[:, c, :]
                        nc.tensor.matmul(ys, ot[0][:, t * P:(t + 1) * P],
                                         wo[:, 0, 512 * c:512 * (c + 1)],
                                         start=True, stop=False)
                        nc.tensor.matmul(ys, ot[1][:, t * P:(t + 1) * P],
                                         wo[:, 1, 512 * c:512 * (c + 1)],
                                         start=False, stop=True)
                        if tail and c == 0:
                            # ACT is exp-idle in the drain tail
                            nc.scalar.copy(ysb[:, 512 * c:512 * (c + 1)], ys)
                        else:
                            nc.vector.tensor_copy(
                                ysb[:, 512 * c:512 * (c + 1)], ys)
                    nc.sync.dma_start(y_d[t * P:(t + 1) * P, :], ysb[:])

                for j in range(T):
                    for h in range(HPC):
                        p_, hl = divmod(h, 2)
                        moff = wdone[(h, j)] if j < WJ else 128 * j
                        if moff >= L:
                            continue
                        scores_exp(ps_s, 1024, p_, hl, j, moff, L - moff,
                                   j >= WJ, ps[h][j][:], moff - 128 * j)
                    if j >= 2:
                        for h in range(HPC):
                            emit_o(h, j - 2)
                            norm_dve(h, j - 2)
                    if j >= 3:
                        for p_ in range(2):
                            norm_pe(p_, j - 3)
                    if j >= 5:
                        emit_y(j - 5)
                # drain, ordered by dependency readiness so the in-order
                # PE queue never parks behind a DVE wait: both remaining
                # O-groups run back-to-back (the T-1 group borrows a bank
                # from the now-idle scores pool), transposes and the last
                # y tiles after
                obt2 = ps_s.tile([P, 4, 128], F32, tag="s", name="obt2")
                emit_y(T - 5, tail=True)
                emit_y(T - 4, tail=True)
                for h in range(HPC):
                    emit_o(h, T - 2)
                    norm_dve(h, T - 2)
                for h in range(HPC):
                    for j in range(T):
                        nc.tensor.matmul(
                            obt2[:, h, 0:65],
                            ps[h][j][:, 128 * (T - 1 - j):128 * (T - 1 - j) + P],
                            vb[j][:, h, :], start=(j == 0), stop=(j == T - 1))
                    p_, hl = divmod(h, 2)
                    rec = nrm_p.tile([P, 1], F32, tag="rec")
                    nc.vector.reciprocal(rec[:], obt2[:, h, 64:65])
                    if hl == 0:
                        osbt[p_][T - 1] = osb_p.tile([P, P], F16, tag="osb",
                                                     name="osb")
                    nc.vector.tensor_scalar(
                        osbt[p_][T - 1][:, 64 * hl:64 * hl + 64],
                        obt2[:, h, 0:64], rec[:], None, ALU.mult)
                for p_ in range(2):
                    norm_pe(p_, T - 3)
                emit_y(T - 3, tail=True)
                for p_ in range(2):
                    norm_pe(p_, T - 2)
                emit_y(T - 2, tail=True)
                for p_ in range(2):
                    norm_pe(p_, T - 1)
                emit_y(T - 1, tail=True)

    # Pin the activation-table chooser to natural_log_exp_and_others (which
    # serves every ACT func used here: Copy/Square/Ln/Exp).  The insertion
    # pass picks the first set containing each func, which thrashes between
    # sets; masking the others (indices preserved, so the emitted
    # act_func_set_id still refers to the right act_info.json entry) yields
    # one table load total.
    import concourse.bacc as _bacc_mod
    _orig_tables = _bacc_mod.get_activation_tables
    _KEEP = "natural_log_exp_and_others"

    def _pinned_tables(arch):
        tabs = _orig_tables(arch)
        assert _KEEP in tabs
        return {name: (s if name == _KEEP else set()) for name, s in tabs.items()}

    _bacc_mod.get_activation_tables = _pinned_tables
    try:
        nc.compile()
    finally:
        _bacc_mod.get_activation_tables = _orig_tables
    return nc


_NC = None


def _get_nc():
    global _NC
    if _NC is None:
        _NC = build_nc()
    return _NC


def _center(w):
    # fold LayerNorm mean-subtraction into the projection weights (per head)
    w3 = w.astype(np.float64).reshape(-1, D, EMB)
    w3 = w3 - w3.mean(axis=1, keepdims=True)
    return w3.reshape(-1, EMB)


def make_in_maps(x, Wq, Wk, Wv, gq, bq, gk, bk, Wo):
    x = np.asarray(x, np.float32)
    Wq = np.asarray(Wq, np.float32)
    Wk = np.asarray(Wk, np.float32)
    Wv = np.asarray(Wv, np.float32)
    Wo = np.asarray(Wo, np.float32)
    gq = np.asarray(gq, np.float32)
    bq = np.asarray(bq, np.float32)
    gk = np.asarray(gk, np.float32)
    bk = np.asarray(bk, np.float32)

    ident = np.eye(P, dtype=np.float16)
    # additive causal mask for the diagonal 128x128 block of ST[m, l_local]:
    # invalid where l < m.  -28672 underflows exp() to 0 since valid scores
    # are bounded by |q||k| <= 64.
    maskf = np.where(np.arange(P)[None, :] < np.arange(P)[:, None], -28672.0, 0.0
                     ).astype(np.float16)
    im = np.ascontiguousarray(np.stack([ident, maskf], axis=1))  # [P, 2, P]
    gb = np.stack([np.tile(gq, 2), np.tile(bq, 2), np.tile(gk, 2), np.tile(bk, 2)],
                  axis=1).astype(np.float32)  # [128, 4]

    in_maps = []
    for c in range(NCORES):
        n, g = divmod(c, HPC)
        rows = slice(256 * g, 256 * (g + 1))
        xT = np.ascontiguousarray(x[n].T.reshape(E, P, L)).astype(np.float16)
        wqT = _center(Wq[rows]).T.reshape(E, P, 256)
        wkT = _center(Wk[rows]).T.reshape(E, P, 256)
        wqk = np.concatenate([wqT, wkT], axis=2).astype(np.float16)
        wvT = Wv[rows].astype(np.float64).T.reshape(E, P, 256).astype(np.float16)
        woT = Wo[:, rows].T.reshape(2, P, EMB).transpose(1, 0, 2).astype(np.float16)
        in_maps.append({
            "xT": xT, "wqk": np.ascontiguousarray(wqk),
            "wv": np.ascontiguousarray(wvT), "wo": np.ascontiguousarray(woT),
            "im": im, "gb": gb,
        })
    return in_maps


def kernel(x, mask, Wq, Wk, Wv, gq, bq, gk, bk, Wo, bo):
    nc = _get_nc()
    in_maps = make_in_maps(x, Wq, Wk, Wv, gq, bq, gk, bk, Wo)
    res = run_bass_kernel_spmd(nc, in_maps, list(range(NCORES)))
    bo = np.asarray(bo, np.float32)
    y = np.zeros((2, L, EMB), np.float32)
    for n in range(2):
        acc = np.zeros((L, EMB), np.float32)
        for g in range(HPC):
            r = res.results[HPC * n + g]
            acc += r["y"].astype(np.float32)
        y[n] = acc + bo[None, :]
    return y



# revision 45
# speedup vs baseline: 1.0093x; 1.0093x over previous
"""Trainium2 Bass kernel for CustomMultiHeadSelfAttention (fused q/k LayerNorm).

Reference computation (per batch n):
    q = x @ Wq.T ; k = x @ Wk.T ; v = x @ Wv.T          (split into 16 heads of 64)
    q = LN_head(q) * gq + bq ; k = LN_head(k) * gk + bk  (LayerNorm over head_dim)
    out = causal_softmax(q @ k.T) @ v                    (per head)
    y = concat_heads(out) @ Wo.T + bo

Sharding: 8 cores = 2 batches x 4 head-groups (4 heads each).  Each core
computes its heads' attention and a partial y = out_heads @ Wo[:, cols].T;
the host sums the 4 partials per batch and adds bo.

Device dataflow (all 16-bit operands fp16 except P=exp(S) which is bf16 for
range; PSUM accumulation fp32):
  - Projections stream fp16 xT/W chunks; LN mean is folded into centered
    Q/K weights so LN reduces to q * rsqrt(mean(q^2)+eps); rstd is
    broadcast per head-group on GPSIMD and applied with one tensor_tensor;
    *g+b is fused into the post-transpose PSUM->SBUF store.
  - Scores are computed transposed per head, ST[m, l] = k_ln q_ln^T, with a
    fp16 -3e4 additive mask on the diagonal 128x128 block via an identity
    matmul; exp() on ScalarE (no max subtraction: LN bounds |score| <= 64).
  - Overlap: for m-blocks j<8 the l<1024 part of the scores ("window") is
    emitted between the two projection halves so ScalarE exps run under the
    second projection half; P tiles are stashed in SBUF.
  - O is accumulated P-stationary per l-tile: O[l,65] += P[m,l-block]^T
    V_aug[m,65] (V has a ones column -> col 64 = softmax denominator), so
    the moving operand is only 65 wide.  Slot t is normalized (per-partition
    reciprocal) as soon as j reaches t, transposed back to OT layout on the
    PE, and written to the fc-out stationary tile.
  - Output projection y[t] = sum_p OT_p[:,t]^T @ WoT_p is emitted per
    l-tile inside the last head's pass, so it overlaps attention.
"""

import numpy as np

import concourse.bass as bass
import concourse.tile as tile
from concourse import bacc, mybir
from concourse.bass_utils import run_bass_kernel_spmd

F32 = mybir.dt.float32
F32R = mybir.dt.float32r
F16 = mybir.dt.float16
BF16 = mybir.dt.bfloat16
F8E5 = mybir.dt.float8e5
U8 = mybir.dt.uint8
DR = mybir.MatmulPerfMode.DoubleRow

P = 128
EMB = 1024
L = 2048
D = 64
HPC = 4           # heads per core
NCORES = 8
EPS = 1e-5
T = L // P        # 16 l-tiles
E = EMB // P      # 8 emb chunks
WJ = 8            # window m-blocks: j<8 cover l in [128j, 1024)
WEND = 1024
USE_DR_MASK = False
AF = mybir.ActivationFunctionType
ALU = mybir.AluOpType


def _subs(off, ln):
    """Split [off, off+ln) at 512-multiples (PSUM bank boundaries)."""
    out = []
    cur = off
    while cur < off + ln:
        nxt = min((cur // 512 + 1) * 512, off + ln)
        out.append((cur, nxt - cur))
        cur = nxt
    return out


def build_nc():
    nc = bacc.Bacc("TRN2", target_bir_lowering=False, debug=False, num_devices=NCORES)

    xT_d = nc.dram_tensor("xT", [E, P, L], F16, kind="ExternalInput")
    wqk_d = nc.dram_tensor("wqk", [E, P, 512], F16, kind="ExternalInput")
    wv_d = nc.dram_tensor("wv", [E, P, 256], F16, kind="ExternalInput")
    wo_d = nc.dram_tensor("wo", [P, 2, EMB], F16, kind="ExternalInput")
    im_d = nc.dram_tensor("im", [P, 2, P], F16, kind="ExternalInput")  # ident|mask
    gb_d = nc.dram_tensor("gb", [P, 4], F32, kind="ExternalInput")  # gq2 bq2 gk2 bk2
    y_d = nc.dram_tensor("y", [L, EMB], F16, kind="ExternalOutput")

    with tile.TileContext(nc) as tc:
        with (
            tc.tile_pool(name="const", bufs=1) as const_p,
            tc.tile_pool(name="vbuf", bufs=1) as vbuf_p,
            tc.tile_pool(name="qtkt", bufs=1) as qtkt_p,
            tc.tile_pool(name="stash", bufs=1) as stash_p,
        ):
            im = const_p.tile([P, 2, P], F16, tag="im")
            gb = const_p.tile([P, 4], F32, tag="gb")
            epst = const_p.tile([P, 1], F32, tag="epst")
            nc.vector.memset(epst[:], EPS)
            ident = im[:, 0, :]
            maskf = im[:, 1, :]

            # V with a ones column per head: vb[t][:, h, 0:65]
            vb = []
            for t in range(T):
                v_ = vbuf_p.tile([P, HPC, 65], BF16, tag=f"vb{t}", name=f"vb{t}")
                # only col 64 (the denominator ones-column) needs the memset;
                # DVE so the Pool DGE queue stays free for the input DMAs
                nc.vector.memset(v_[:, :, 64:65], 1.0)
                vb.append(v_)

            # QT/KT: partitions = d of head pair (rows [0:64]=head 2p, [64:128]
            # = head 2p+1); dims [pair, l]
            qt = qtkt_p.tile([P, 2, L], F16, tag="qt", name="qt")
            kt = qtkt_p.tile([P, 2, L], F16, tag="kt", name="kt")

            # persistent P stash: ps[h][j] covers l in [128j, 2048) at col
            # l-128j.  j<8 tiles live for the whole kernel (their l<1024
            # window part is exp'd during phase 1); j>=8 tiles are allocated
            # inside the phase-2 pool block to keep the phase-1 footprint
            # under the SBUF limit.
            ps = [[stash_p.tile([P, L - 128 * j], BF16, tag=f"ps{h}_{j}",
                                name=f"ps{h}_{j}") for j in range(WJ)] + [None] * (T - WJ)
                  for h in range(HPC)]

            def scores_exp(pool, width, p_, hl, j, off, ln, diag, out16, ocol):
                """ST[m, l] for m-block j over l in [off, off+ln), exp into
                out16[:, ocol:ocol+ln]."""
                rows = slice(64 * hl, 64 * hl + 64)
                base = (off // width) * width if width == 512 else 512 * (off // 512)
                sps = pool.tile([P, width], F32, tag="s")
                for (soff, sln) in _subs(off, ln):
                    d_ = diag and soff == off
                    nc.tensor.matmul(
                        sps[:, soff - base:soff - base + sln],
                        kt[rows, p_, j * P:(j + 1) * P],
                        qt[rows, p_, soff:soff + sln],
                        start=True, stop=not d_)
                    if d_:
                        nc.tensor.matmul(
                            sps[:, off - base:off - base + P],
                            ident, maskf, start=False, stop=True)
                nc.scalar.activation(
                    out16[:, ocol:ocol + ln],
                    sps[:, off - base:off - base + ln], AF.Exp)

            # ============ Phase 1: projections + LN + transpose ==========
            # Q|K projections run first (the LN/transpose chain and the
            # attention window depend on them); V projections are deferred
            # to the back half and past the phase boundary, where they give
            # the PE useful filler while the window/first main exps run.
            with (
                tc.tile_pool(name="xt", bufs=1) as xt_p,
                tc.tile_pool(name="wts", bufs=1) as wts_p,
                tc.tile_pool(name="rows", bufs=4) as rows_p,
                tc.tile_pool(name="stats", bufs=4) as stats_p,
                tc.tile_pool(name="sq", bufs=3) as sq_p,
                tc.tile_pool(name="rsb", bufs=3) as rsb_p,
                tc.tile_pool(name="ps_qk", bufs=3, space="PSUM") as ps_qk,
                tc.tile_pool(name="ps_v", bufs=1, space="PSUM") as ps_v,
                tc.tile_pool(name="ps_tr", bufs=1, space="PSUM") as ps_tr,
                tc.tile_pool(name="ps_sw", bufs=2, space="PSUM") as ps_sw,
            ):
                # e-chunked loads round-robined over the SP / ACT / gpsimd
                # DGE queues: parallel issue + parallel DMA engines so the
                # first qk-projection step can start ~2.5us in
                wqk = wts_p.tile([P, E, 512], F16, tag="wqk", name="wqk")
                wv = wts_p.tile([P, E, 256], F16, tag="wv", name="wv")
                xq = [xt_p.tile([P, E, 512], F16, tag=f"xq{q}", name=f"xq{q}")
                      for q in range(4)]
                for ep in range(4):
                    es = slice(2 * ep, 2 * ep + 2)
                    nc.sync.dma_start(
                        wqk[:, es, :], wqk_d[es].rearrange("e p c -> p e c"))
                    nc.scalar.dma_start(
                        xq[0][:, es, :],
                        xT_d[es, :, 0:512].rearrange("e p c -> p e c"))
                nc.sync.dma_start(im[:], im_d[:])
                nc.sync.dma_start(gb[:], gb_d[:])
                # order the Pool DGE queue behind the first wave (the DMA
                # bandwidth is serial: a big early xq1 transfer would starve
                # the first qk steps), then chunk the second quarter
                dgate = rows_p.tile([P, 1], F16, tag="dgate", name="dgate")
                nc.gpsimd.tensor_copy(dgate[:], xq[0][:, 7, 511:512])
                for ep in range(4):
                    es = slice(2 * ep, 2 * ep + 2)
                    nc.gpsimd.dma_start(
                        xq[1][:, es, :],
                        xT_d[es, :, 512:1024].rearrange("e p c -> p e c"))
                for q in range(2, 4):
                    eng = (nc.sync, nc.scalar)[q - 2]
                    eng.dma_start(
                        xq[q][:], xT_d[:, :, 512 * q:512 * (q + 1)].rearrange(
                            "e p c -> p e c"))
                nc.gpsimd.dma_start(wv[:], wv_d.rearrange("e p c -> p e c"))

                # p-state warmup: ~26 junk matmuls keep the PE continuously
                # busy from ~0.4us until the first input chunks land, so the
                # first projection (and everything after) runs at 2.4 GHz
                # instead of spending its first 3us at 1.2.
                warm_sb = rows_p.tile([P, 256], F16, tag="warm", name="warm")
                nc.vector.memset(warm_sb[:], 0.0)
                warm_ps = ps_sw.tile([P, 512], F32, tag="s", name="warm_ps")
                for _ in range(18):
                    nc.tensor.matmul(warm_ps[:, 0:256], warm_sb[:, 0:128],
                                     warm_sb[:], start=True, stop=True)

                pvt = ps_v.tile([P, 2, 256], F32, tag="pv")
                trt = ps_tr.tile([P, 2, 2, 2, P], F16, tag="tr")  # [t%2,qk,pair]
                rowt = [None] * T  # (qrow, krow) awaiting lag-2 transpose
                chain = [None] * T  # (pqk, rsb) awaiting lag-1 rstd-mult

                def proj_flush(t):
                    # transposes + qt/kt stores for step t (*g+b fused)
                    qrow, krow = rowt[t]
                    s = t % 2
                    for p_ in range(2):
                        nc.tensor.transpose(
                            trt[:, s, 0, p_, :],
                            qrow[:, 128 * p_:128 * (p_ + 1)], ident)
                        nc.tensor.transpose(
                            trt[:, s, 1, p_, :],
                            krow[:, 128 * p_:128 * (p_ + 1)], ident)
                    nc.vector.tensor_scalar(
                        qt[:, :, t * P:(t + 1) * P], trt[:, s, 0, :, :],
                        gb[:, 0:1], gb[:, 1:2], ALU.mult, ALU.add)
                    nc.vector.tensor_scalar(
                        kt[:, :, t * P:(t + 1) * P], trt[:, s, 1, :, :],
                        gb[:, 2:3], gb[:, 3:4], ALU.mult, ALU.add)

                def ln_tail(t):
                    # rstd-multiply of step t, emitted one step late so the
                    # in-order DVE queue never blocks waiting the ACT/Pool
                    # chain (which would serialize consecutive steps)
                    pqk, rsb = chain[t]
                    qrow = rows_p.tile([P, 256], F16, tag="qrow")
                    krow = rows_p.tile([P, 256], F16, tag="krow")
                    nc.vector.tensor_tensor(
                        qrow[:], pqk[:, 0:256], rsb[:, 0:256], ALU.mult)
                    nc.vector.tensor_tensor(
                        krow[:], pqk[:, 256:512], rsb[:, 256:512], ALU.mult)
                    rowt[t] = (qrow, krow)

                def proj_qk(t):
                    if t >= 2:
                        proj_flush(t - 2)
                    pqk = ps_qk.tile([P, 512], F32, tag="pqk")
                    for e in range(E):
                        xch = xq[t // 4][:, e, (t % 4) * P:(t % 4 + 1) * P]
                        nc.tensor.matmul(pqk[:], xch, wqk[:, e, :],
                                         start=(e == 0), stop=(e == E - 1))
                    # LN stats: sum of squares per (l, head) for q and k
                    sq = sq_p.tile([P, 512], F32, tag="sq")
                    nc.scalar.activation(sq[:], pqk[:], AF.Square)
                    ssq = stats_p.tile([P, 8], F32, tag="ssq")
                    nc.vector.tensor_reduce(
                        ssq[:], sq[:].rearrange("p (g d) -> p g d", d=D),
                        axis=mybir.AxisListType.X, op=ALU.add)
                    # rstd = (ssq/64 + eps)^-0.5 as exp(-0.5*ln(v)); with Ln
                    # instead of Sqrt every ACT func in the kernel (Square/
                    # Ln/Exp/Copy) lives in the natural_log_exp_and_others
                    # table set -> a single table load for the whole kernel
                    lnv = stats_p.tile([P, 8], F32, tag="lnv")
                    nc.scalar.activation(lnv[:], ssq[:], AF.Ln,
                                         bias=epst[:], scale=1.0 / D)
                    rstd = stats_p.tile([P, 8], F32, tag="rstd")
                    nc.scalar.activation(rstd[:], lnv[:], AF.Exp, scale=-0.5)
                    # broadcast rstd across each 64-col head group (GPSIMD)
                    rsb = rsb_p.tile([P, 512], F32, tag="rsb")
                    nc.gpsimd.tensor_copy(
                        rsb[:].rearrange("p (g d) -> p g d", d=D),
                        rstd[:].broadcast_to([P, 8, D]))
                    chain[t] = (pqk, rsb)
                    if t >= 1:
                        ln_tail(t - 1)

                def proj_v(t):
                    pv = pvt[:, t % 2, :]
                    for e in range(E):
                        xch = xq[t // 4][:, e, (t % 4) * P:(t % 4 + 1) * P]
                        nc.tensor.matmul(pv, xch, wv[:, e, :],
                                         start=(e == 0), stop=(e == E - 1))
                    # V -> vb[t] (strided into 65-wide head slots); DVE so
                    # the ACT queue stays exp-only here
                    nc.vector.tensor_copy(
                        vb[t][:, :, 0:64],
                        pv.rearrange("p (h d) -> p h d", h=HPC))

                # greedy window plan: after qk-step t (which flushes tile
                # t-2) qt/kt cols < 128*(t-1) exist; emit ready score
                # pieces (h, j<8, any l) so the ACT exp stream starts as
                # soon as the first tiles are flushed and never starves
                # during the V-projection tail.  Budgets approximate the
                # per-step ACT headroom next to the LN-stats chain.
                def plan_window():
                    pend = {(h, j): 128 * j
                            for h in range(HPC) for j in range(WJ)}
                    order = [(h, j) for j in range(WJ) for h in range(HPC)]
                    nsteps = T + (T - 6)
                    sched = {t: [] for t in range(nsteps)}
                    for t in range(3, nsteps):
                        limit = min(128 * (t - 1), L) if t < T else L
                        budget = 1300 if t < 10 else (1700 if t < T else 700)
                        # tier 1: the l<1024 window (unblocks nothing later
                        # but is all that exists early); tier 2: l>=1024
                        for cap in (WEND, L):
                            for (h, j) in order:
                                while budget > 0:
                                    l0 = pend[(h, j)]
                                    if l0 >= min(limit, cap):
                                        break
                                    l1 = min(limit, cap, l0 + 512,
                                             (l0 // 512 + 1) * 512)
                                    sched[t].append(
                                        (h, j, l0, l1, l0 == 128 * j))
                                    pend[(h, j)] = l1
                                    budget -= l1 - l0
                                if budget <= 0:
                                    break
                            if budget <= 0:
                                break
                    assert all(v >= WEND for v in pend.values()), pend
                    return sched, pend

                wsched, wdone = plan_window()

                def emit_pieces(t):
                    for (h, j, l0, l1, dg) in wsched.get(t, ()):
                        p_, hl = divmod(h, 2)
                        scores_exp(ps_sw, 512, p_, hl, j, l0, l1 - l0, dg,
                                   ps[h][j][:], l0 - 128 * j)

                for t in range(T):
                    proj_qk(t)
                    emit_pieces(t)
                    if t >= 10:
                        proj_v(t - 10)
                ln_tail(T - 1)
                proj_flush(T - 2)
                proj_flush(T - 1)
                # V tail interleaved with remaining window pieces: the PE
                # and ACT both stay busy across the phase boundary
                for i, tv in enumerate(range(6, T)):
                    proj_v(tv)
                    emit_pieces(T + i)

            # ================= Phase 2: attention + fc_out ===============
            # All 4 heads interleave at the j (m-block) level: per group j
            # every head emits scores+exp(j), then O/normalize for l-tile
            # j-2, pair transposes for j-3, and the SINGLE shared fc_out
            # tile j-5 (both pairs accumulate in PSUM).  This spreads the
            # DVE normalize/copy work and the fc_out evenly over the whole
            # phase instead of piling it into per-head tails.
            with (
                tc.tile_pool(name="stash2", bufs=1) as stash2_p,
                tc.tile_pool(name="otb", bufs=1) as ot_p,
                tc.tile_pool(name="wo", bufs=1) as wo_p,
                tc.tile_pool(name="osb", bufs=6) as osb_p,
                tc.tile_pool(name="nrm", bufs=8) as nrm_p,
                tc.tile_pool(name="ysb", bufs=3) as ysb_p,
                tc.tile_pool(name="ps_s", bufs=2, space="PSUM") as ps_s,
                tc.tile_pool(name="ps_o", bufs=1, space="PSUM") as ps_o,
                tc.tile_pool(name="ps_t2", bufs=1, space="PSUM") as ps_t2,
                tc.tile_pool(name="ps_y", bufs=1, space="PSUM") as ps_y,
            ):
                for h in range(HPC):
                    for j in range(WJ, T):
                        ps[h][j] = stash2_p.tile(
                            [P, L - 128 * j], BF16, tag=f"ps{h}_{j}",
                            name=f"ps{h}_{j}")
                ot = [ot_p.tile([P, L], F16, tag=f"ot{p_}", name=f"ot{p_}")
                      for p_ in range(2)]
                wo = wo_p.tile([P, 2, EMB], F16, tag="wo")
                nc.sync.dma_start(wo[:], wo_d[:])

                # PSUM: 4 banks scores (2x1024 dbl-buf), 1 bank holds the 4
                # per-head O slots (start=True zeroing only clobbers OPEN
                # accumulation groups in a bank; closed data is read-safe),
                # 1 bank transpose scratch, 2 banks fc_out double-buffer.
                obt = ps_o.tile([P, 4, 128], F32, tag="obt")
                trt2 = ps_t2.tile([P, 2, P], F16, tag="trt2")
                yp2 = ps_y.tile([P, 2, 512], F32, tag="yp2")

                def oslot(h):
                    return obt[:, h, 0:65]

                def trs(i):
                    return trt2[:, i, :]

                osbt = [[None] * T for _ in range(2)]

                def emit_o(h, t):
                    # one l-tile slot: contiguous accumulation over j<=t
                    for j in range(t + 1):
                        nc.tensor.matmul(
                            oslot(h),
                            ps[h][j][:, 128 * (t - j):128 * (t - j) + P],
                            vb[j][:, h, :], start=(j == 0), stop=(j == t))

                def norm_dve(h, t):
                    p_, hl = divmod(h, 2)
                    rec = nrm_p.tile([P, 1], F32, tag="rec")
                    nc.vector.reciprocal(rec[:], oslot(h)[:, 64:65])
                    if hl == 0:
                        osbt[p_][t] = osb_p.tile([P, P], F16, tag="osb",
                                                 name="osb")
                    nc.vector.tensor_scalar(
                        osbt[p_][t][:, 64 * hl:64 * hl + 64],
                        oslot(h)[:, 0:64], rec[:], None, ALU.mult)

                def norm_pe(p_, t):
                    nc.tensor.transpose(trs(p_), osbt[p_][t][:], ident)
                    nc.vector.tensor_copy(
                        ot[p_][:, t * P:(t + 1) * P], trs(p_))

                def emit_y(t, tail=False):
                    ysb = ysb_p.tile([P, EMB], F16, tag="ysb")
                    for c in range(2):
                        ys = yp2[:, c, :]
                        nc.tensor.matmul(ys, ot[0][:, t * P:(t + 1) * P],
                                         wo[:, 0, 512 * c:512 * (c + 1)],
                                         start=True, stop=False)
                        nc.tensor.matmul(ys, ot[1][:, t * P:(t + 1) * P],
                                         wo[:, 1, 512 * c:512 * (c + 1)],
                                         start=False, stop=True)
                        if tail and c == 0:
                            # ACT is exp-idle in the drain tail
                            nc.scalar.copy(ysb[:, 512 * c:512 * (c + 1)], ys)
                        else:
                            nc.vector.tensor_copy(
                                ysb[:, 512 * c:512 * (c + 1)], ys)
                    nc.sync.dma_start(y_d[t * P:(t + 1) * P, :], ysb[:])

                for j in range(T):
                    for h in range(HPC):
                        p_, hl = divmod(h, 2)
                        moff = wdone[(h, j)] if j < WJ else 128 * j
                        if moff >= L:
                            continue
                        scores_exp(ps_s, 1024, p_, hl, j, moff, L - moff,
                                   j >= WJ, ps[h][j][:], moff - 128 * j)
                    if j >= 2:
                        for h in range(HPC):
                            emit_o(h, j - 2)
                            norm_dve(h, j - 2)
                    if j >= 3:
                        for p_ in range(2):
                            norm_pe(p_, j - 3)
                    if j >= 5:
                        emit_y(j - 5)
                # drain, ordered by dependency readiness so the in-order
                # PE queue never parks behind a DVE wait: both remaining
                # O-groups run back-to-back (the T-1 group borrows a bank
                # from the now-idle scores pool), transposes and the last
                # y tiles after
                obt2 = ps_s.tile([P, 4, 128], F32, tag="s", name="obt2")
                emit_y(T - 5, tail=True)
                emit_y(T - 4, tail=True)
                for h in range(HPC):
                    emit_o(h, T - 2)
                    norm_dve(h, T - 2)
                for h in range(HPC):
                    for j in range(T):
                        nc.tensor.matmul(
                            obt2[:, h, 0:65],
                            ps[h][j][:, 128 * (T - 1 - j):128 * (T - 1 - j) + P],
                            vb[j][:, h, :], start=(j == 0), stop=(j == T - 1))
                    p_, hl = divmod(h, 2)
                    rec = nrm_p.tile([P, 1], F32, tag="rec")
                    nc.vector.reciprocal(rec[:], obt2[:, h, 64:65])
                    if hl == 0:
                        osbt[p_][T - 1] = osb_p.tile([P, P], F16, tag="osb",
                                                     name="osb")
                    nc.vector.tensor_scalar(
                        osbt[p_][T - 1][:, 64 * hl:64 * hl + 64],
                        obt2[:, h, 0:64], rec[:], None, ALU.mult)
                for p_ in range(2):
                    norm_pe(p_, T - 3)
                emit_y(T - 3, tail=True)
                for p_ in range(2):
                    norm_pe(p_, T - 2)
                emit_y(T - 2, tail=True)
                for p_ in range(2):
                    norm_pe(p_, T - 1)
                emit_y(T - 1, tail=True)

    # Pin the activation-table chooser to natural_log_exp_and_others (which
    # serves every ACT func used here: Copy/Square/Ln/Exp).  The insertion
    # pass picks the first set containing each func, which thrashes between
    # sets; masking the others (indices preserved, so the emitted
    # act_func_set_id still refers to the right act_info.json entry) yields
    # one table load total.
    import concourse.bacc as _bacc_mod
    _orig_tables = _bacc_mod.get_activation_tables
    _KEEP = "natural_log_exp_and_others"

    def _pinned_tables(arch):
        tabs = _orig_tables(arch)
        assert _KEEP in tabs
        return {name: (s if name == _KEEP else set()) for name, s in tabs.items()}

    _bacc_mod.get_activation_tables = _pinned_tables
    try:
        nc.compile()
    finally:
        _bacc_mod.get_activation_tables = _orig_tables
    return nc


_NC = None


def _get_nc():
    global _NC
    if _NC is None:
        _NC = build_nc()
    return _NC


def _center(w):
    # fold LayerNorm mean-subtraction into the projection weights (per head)
    w3 = w.astype(np.float64).reshape(-1, D, EMB)
    w3 = w3 - w3.mean(axis=1, keepdims=True)
    return w3.reshape(-1, EMB)


def make_in_maps(x, Wq, Wk, Wv, gq, bq, gk, bk, Wo):
    x = np.asarray(x, np.float32)
    Wq = np.asarray(Wq, np.float32)
    Wk = np.asarray(Wk, np.float32)
    Wv = np.asarray(Wv, np.float32)
    Wo = np.asarray(Wo, np.float32)
    gq = np.asarray(gq, np.float32)
    bq = np.asarray(bq, np.float32)
    gk = np.asarray(gk, np.float32)
    bk = np.asarray(bk, np.float32)

    ident = np.eye(P, dtype=np.float16)
    # additive causal mask for the diagonal 128x128 block of ST[m, l_local]:
    # invalid where l < m.  -28672 underflows exp() to 0 since valid scores
    # are bounded by |q||k| <= 64.
    maskf = np.where(np.arange(P)[None, :] < np.arange(P)[:, None], -28672.0, 0.0
                     ).astype(np.float16)
    im = np.ascontiguousarray(np.stack([ident, maskf], axis=1))  # [P, 2, P]
    gb = np.stack([np.tile(gq, 2), np.tile(bq, 2), np.tile(gk, 2), np.tile(bk, 2)],
                  axis=1).astype(np.float32)  # [128, 4]

    in_maps = []
    for c in range(NCORES):
        n, g = divmod(c, HPC)
        rows = slice(256 * g, 256 * (g + 1))
        xT = np.ascontiguousarray(x[n].T.reshape(E, P, L)).astype(np.float16)
        wqT = _center(Wq[rows]).T.reshape(E, P, 256)
        wkT = _center(Wk[rows]).T.reshape(E, P, 256)
        wqk = np.concatenate([wqT, wkT], axis=2).astype(np.float16)
        wvT = Wv[rows].astype(np.float64).T.reshape(E, P, 256).astype(np.float16)
        woT = Wo[:, rows].T.reshape(2, P, EMB).transpose(1, 0, 2).astype(np.float16)
        in_maps.append({
            "xT": xT, "wqk": np.ascontiguousarray(wqk),
            "wv": np.ascontiguousarray(wvT), "wo": np.ascontiguousarray(woT),
            "im": im, "gb": gb,
        })
    return in_maps


def kernel(x, mask, Wq, Wk, Wv, gq, bq, gk, bk, Wo, bo):
    nc = _get_nc()
    in_maps = make_in_maps(x, Wq, Wk, Wv, gq, bq, gk, bk, Wo)
    res = run_bass_kernel_spmd(nc, in_maps, list(range(NCORES)))
    bo = np.asarray(bo, np.float32)
    y = np.zeros((2, L, EMB), np.float32)
    for n in range(2):
        acc = np.zeros((L, EMB), np.float32)
        for g in range(HPC):
            r = res.results[HPC * n + g]
            acc += r["y"].astype(np.float32)
        y[n] = acc + bo[None, :]
    return y

